# revision 22
# baseline (speedup 1.0000x reference)
"""Trainium2 Bass kernel for nn_Attn_spa (dense transformer attention with
pre-computed bias logits), SPMD over 8 NeuronCores.

Sharding: core c handles batch b = c//2 and head-half hh = c%2 (8 of 16 heads).
Per-core device program (all layouts keep seq as the free dim):
  preT = silu(Wpre.T @ xT + bpre)            [C,N]   (transposed pre)
  qT/kT = (Wq_h.T/8) @ xT                    [512,N]
  v    = xT.T @ Wv_h                         [N,512] (natural)
  L    = preT.T @ preT                       [N,N]   (bias logits, symmetric)
  per head h: sT = kT_h.T @ qT_h             [m,n]  (scoreT tile-by-tile)
              u  = exp(pi/32*L + sT)         (unnormalized attn, transposed)
              uo[d,n] += v_h[m,d].T @ u ; den[n] += 1.T @ u
              outT_h = uo * (1/den)          (broadcast via ones-matmul)
  y_partial = outT.T @ Wproj_h               [N,C]
Host: y[b] = y_partial(core 2b) + y_partial(core 2b+1) + x[b] + bproj.

Matmuls run as float32r (full-rate fp32 on the PE; inputs bitcast) except the
tiny reciprocal-broadcast / pi-broadcast matmuls which stay exact fp32.
"""

import sys

sys.path.insert(0, "/opt/trn_rl_repo")

import numpy as np

B, N, C = 4, 1024, 1024
H, DH = 16, 64
NCORES = 8
CH = C // 2  # features per core in the head-sharded dim (8 heads * 64)

USE_F32R = True

_cached = {}


def _build_nc():
    import concourse.bass as bass
    import concourse.mybir as mybir
    import concourse.tile as tile
    from concourse import bacc

    f32 = mybir.dt.float32
    f32r = mybir.dt.float32r
    AF = mybir.ActivationFunctionType
    ALU = mybir.AluOpType

    mmdt = f32r if USE_F32R else f32

    nc = bacc.Bacc("TRN2", target_bir_lowering=False, debug=False)

    xt_d = nc.dram_tensor("xt", [C, N], mmdt, kind="ExternalInput")
    wpre_d = nc.dram_tensor("wpre", [C, C], mmdt, kind="ExternalInput")
    wq_d = nc.dram_tensor("wq", [C, CH], mmdt, kind="ExternalInput")
    wk_d = nc.dram_tensor("wk", [C, CH], mmdt, kind="ExternalInput")
    wv_d = nc.dram_tensor("wv", [C, CH], mmdt, kind="ExternalInput")
    wproj_d = nc.dram_tensor("wproj", [CH, C], mmdt, kind="ExternalInput")
    bpre_d = nc.dram_tensor("bpre", [C], f32, kind="ExternalInput")
    pi_d = nc.dram_tensor("pi", [1, 1], f32, kind="ExternalInput")
    y_d = nc.dram_tensor("y", [N, C], f32, kind="ExternalOutput")

    with tile.TileContext(nc) as tc:
        from contextlib import ExitStack

        with ExitStack() as ctx:
            ppool = ctx.enter_context(tc.tile_pool(name="ps", bufs=1, space="PSUM"))
            work0 = ctx.enter_context(tc.tile_pool(name="work0", bufs=1))

            def chunks(name, n, shape, side="right", dt=None):
                # SBUF pools are per-side LIFO stacks: allocate long-lived
                # tensors on the right, phase-temporaries on the left in
                # reverse-free order.
                tiles, frees = [], []
                for i in range(n):
                    t, f = tc.tile(shape, dt or mmdt, name=f"{name}{i}", side=side)
                    tiles.append(t)
                    frees.append(f)
                return tiles, (lambda fl=frees: [f() for f in reversed(fl)])

            # ---- constants (bottom of the right stack, freed last) ----
            ones32_sb, free_ones32 = tc.tile([128, 128], f32, name="ones32", side="right")
            nc.vector.memset(ones32_sb[:], 1.0)
            ones_sb, free_ones = tc.tile([128, 128], mmdt, name="ones", side="right")
            nc.scalar.copy(ones_sb[:], ones32_sb[:])
            pi_sb, free_pi = tc.tile([1, 1], f32, name="pisb", side="right")
            nc.sync.dma_start(pi_sb[0:1, 0:1], pi_d[:, :])
            bpre_sb, free_bpre = tc.tile([128, 8], f32, name="bpresb", side="right")
            nc.sync.dma_start(bpre_sb[:, :], bpre_d.rearrange("(c p) -> p c", p=128))
            # pi broadcast to all 128 partitions via PE, then * 1/sqrt(C)
            pi_ps = ppool.tile([128, 1], f32, tag="d", bufs=2)
            nc.tensor.matmul(
                pi_ps[:, 0:1], ones32_sb[0:1, 0:128], pi_sb[0:1, 0:1],
                start=True, stop=True,
            )
            pi32_sb, free_pi32 = tc.tile([128, 1], f32, name="pi32", side="right")
            nc.scalar.activation(pi32_sb[:], pi_ps[:], AF.Copy, scale=1.0 / 32.0)

            # ---- load inputs ----
            # left-stack alloc order = reverse free order:
            # pre (freed last) < xt < wq < wk < wv < wpre (freed first)
            pre_sb, free_pre = chunks("pre", 8, [128, N], side="left")
            xt_sb, free_xt = chunks("xt", 8, [128, N], side="left")
            wq_sb, free_wq = chunks("wq", 8, [128, CH], side="left")
            wk_sb, free_wk = chunks("wk", 8, [128, CH], side="left")
            wv_sb, free_wv = chunks("wv", 8, [128, CH], side="left")
            wpre_sb, free_wpre = chunks("wpre", 8, [128, C], side="left")
            for i in range(8):
                nc.sync.dma_start(xt_sb[i][:], xt_d[128 * i : 128 * (i + 1), :])
                nc.sync.dma_start(wpre_sb[i][:], wpre_d[128 * i : 128 * (i + 1), :])
            for i in range(8):
                nc.sync.dma_start(wq_sb[i][:], wq_d[128 * i : 128 * (i + 1), :])
                nc.sync.dma_start(wk_sb[i][:], wk_d[128 * i : 128 * (i + 1), :])
                nc.sync.dma_start(wv_sb[i][:], wv_d[128 * i : 128 * (i + 1), :])

            # ---- phase A: preT = silu(Wpre.T @ xT + bpre) ----
            for co in range(8):
                ps = ppool.tile([128, 1024], f32, tag="s", bufs=2)
                for half in range(2):
                    for ci in range(8):
                        nc.tensor.matmul(
                            ps[:, 512 * half : 512 * (half + 1)],
                            wpre_sb[ci][:, 128 * co : 128 * (co + 1)],
                            xt_sb[ci][:, 512 * half : 512 * (half + 1)],
                            start=(ci == 0), stop=(ci == 7),
                        )
                # silu(z) = z * sigmoid(z), z = psum + bpre  (sim lacks Silu)
                sg = work0.tile([128, 1024], f32, tag="sg", bufs=2)
                nc.scalar.activation(
                    sg[:], ps[:], AF.Sigmoid, bias=bpre_sb[:, co : co + 1]
                )
                nc.vector.scalar_tensor_tensor(
                    pre_sb[co][:], ps[:], bpre_sb[:, co : co + 1], sg[:],
                    ALU.add, ALU.mult,
                )
            free_wpre()

            # ---- phase A2: qT, kT (transposed) and v (natural) ----
            qt_sb, free_qt = chunks("qt", 4, [128, N])  # right side
            kt_sb, free_kt = chunks("kt", 4, [128, N])
            for cq in range(4):
                for dst, w_sb in ((qt_sb, wq_sb), (kt_sb, wk_sb)):
                    ps = ppool.tile([128, 1024], f32, tag="s", bufs=2)
                    for half in range(2):
                        for ci in range(8):
                            nc.tensor.matmul(
                                ps[:, 512 * half : 512 * (half + 1)],
                                w_sb[ci][:, 128 * cq : 128 * (cq + 1)],
                                xt_sb[ci][:, 512 * half : 512 * (half + 1)],
                                start=(ci == 0), stop=(ci == 7),
                            )
                    nc.vector.tensor_copy(dst[cq][:], ps[:])
            v_sb, free_v = chunks("v", 8, [128, CH])
            for nv in range(8):
                ps = ppool.tile([128, 512], f32, tag="u", bufs=2)
                for ci in range(8):
                    nc.tensor.matmul(
                        ps[:],
                        xt_sb[ci][:, 128 * nv : 128 * (nv + 1)],
                        wv_sb[ci][:],
                        start=(ci == 0), stop=(ci == 7),
                    )
                nc.scalar.copy(v_sb[nv][:], ps[:])
            free_wv()
            free_wk()
            free_wq()
            free_xt()

            # ---- phase B: L = preT.T @ preT ----
            l_sb, free_l = chunks("lg", 8, [128, N], dt=f32)
            for m in range(8):
                ps = ppool.tile([128, 1024], f32, tag="s", bufs=2)
                for half in range(2):
                    for c in range(8):
                        nc.tensor.matmul(
                            ps[:, 512 * half : 512 * (half + 1)],
                            pre_sb[c][:, 128 * m : 128 * (m + 1)],
                            pre_sb[c][:, 512 * half : 512 * (half + 1)],
                            start=(c == 0), stop=(c == 7),
                        )
                nc.vector.tensor_copy(l_sb[m][:], ps[:])
            free_pre()

            wproj_sb, free_wproj = chunks("wproj", 4, [128, C])
            for i in range(4):
                nc.sync.dma_start(wproj_sb[i][:], wproj_d[128 * i : 128 * (i + 1), :])
            outt_sb, free_outt = chunks("outt", 4, [128, N])

            # ---- phase D: per-head attention ----
            with tc.tile_pool(name="work", bufs=1) as work:
                for h in range(8):
                    # f32r matmuls require dst partition base 0, and DVE ops
                    # need matching partition bases — so every head computes
                    # in rows 0..63; odd heads DMA-shift into outt rows 64..127.
                    hb = (h % 2) * 64
                    hc = h // 2
                    u_ps = [
                        ppool.tile([128, 512], f32, tag="u", bufs=2, name=f"ups{h}_{t}")
                        for t in range(2)
                    ]
                    d_ps = [
                        ppool.tile([128, 512], f32, tag="d", bufs=2, name=f"dps{h}_{t}")
                        for t in range(2)
                    ]
                    for m in range(8):
                        s_ps = ppool.tile([128, 1024], f32, tag="s", bufs=2)
                        for half in range(2):
                            nc.tensor.matmul(
                                s_ps[:, 512 * half : 512 * (half + 1)],
                                kt_sb[hc][hb : hb + 64, 128 * m : 128 * (m + 1)],
                                qt_sb[hc][hb : hb + 64, 512 * half : 512 * (half + 1)],
                                start=True, stop=True,
                            )
                        # s = (L * pi/32) + s, then u = exp(s)
                        nc.vector.scalar_tensor_tensor(
                            s_ps[:], l_sb[m][:], pi32_sb[:, 0:1], s_ps[:],
                            ALU.mult, ALU.add,
                        )
                        ut = work.tile([128, 1024], mmdt, tag="ut", bufs=4)
                        nc.scalar.activation(ut[:], s_ps[:], AF.Exp)
                        for t in range(2):
                            nc.tensor.matmul(
                                u_ps[t][0:64, :],
                                v_sb[m][:, 64 * h : 64 * (h + 1)],
                                ut[:, 512 * t : 512 * (t + 1)],
                                start=(m == 0), stop=(m == 7),
                            )
                            nc.tensor.matmul(
                                d_ps[t][0:1, :],
                                ones_sb[:, 0:1],
                                ut[:, 512 * t : 512 * (t + 1)],
                                start=(m == 0), stop=(m == 7),
                            )
                    for t in range(2):
                        recip = work.tile([128, 512], f32, tag="rc", bufs=2)
                        nc.vector.reciprocal(recip[0:1, :], d_ps[t][0:1, :])
                        # broadcast 1/den over 64 partitions (fp32 PE matmul)
                        nc.tensor.matmul(
                            d_ps[t][0:64, :],
                            ones32_sb[0:1, 0:64],
                            recip[0:1, :],
                            start=True, stop=True,
                        )
                        bc = work.tile([128, 512], f32, tag="bc", bufs=2)
                        nc.scalar.copy(bc[0:64, :], d_ps[t][0:64, :])
                        if hb == 0:
                            nc.vector.tensor_mul(
                                outt_sb[hc][0:64, 512 * t : 512 * (t + 1)],
                                u_ps[t][0:64, :],
                                bc[0:64, :],
                            )
                        else:
                            shift = work.tile([128, 512], mmdt, tag="sh", bufs=2)
                            nc.vector.tensor_mul(
                                shift[0:64, :], u_ps[t][0:64, :], bc[0:64, :]
                            )
                            nc.sync.dma_start(
                                outt_sb[hc][64:128, 512 * t : 512 * (t + 1)],
                                shift[0:64, :],
                            )

                # ---- phase E: y = outT.T @ Wproj ----
                for mt in range(8):
                    ps = ppool.tile([128, 1024], f32, tag="s", bufs=2)
                    for half in range(2):
                        for cc in range(4):
                            nc.tensor.matmul(
                                ps[:, 512 * half : 512 * (half + 1)],
                                outt_sb[cc][:, 128 * mt : 128 * (mt + 1)],
                                wproj_sb[cc][:, 512 * half : 512 * (half + 1)],
                                start=(cc == 0), stop=(cc == 3),
                            )
                    y_sb = work.tile([128, 1024], f32, tag="y", bufs=3)
                    nc.scalar.copy(y_sb[:], ps[:])
                    nc.sync.dma_start(y_d[128 * mt : 128 * (mt + 1), :], y_sb[:])

            # right stack unwinds in reverse allocation order
            free_outt()
            free_wproj()
            free_l()
            free_v()
            free_kt()
            free_qt()
            free_pi32()
            free_bpre()
            free_pi()
            free_ones()
            free_ones32()

    nc.finalize()
    return nc


def get_nc():
    if "nc" not in _cached:
        _cached["nc"] = _build_nc()
    return _cached["nc"]


def kernel(x, Wq, Wk, Wv, Wproj, bproj, Wpre, bpre, pi):
    x = np.asarray(x, np.float32)
    nc = get_nc()
    in_maps = []
    for c in range(NCORES):
        b, hh = c // 2, c % 2
        sl = slice(CH * hh, CH * (hh + 1))
        in_maps.append(
            {
                "xt": np.ascontiguousarray(x[b].T),
                "wpre": np.asarray(Wpre, np.float32),
                "wq": np.ascontiguousarray(np.asarray(Wq, np.float32)[:, sl]) * 0.125,
                "wk": np.ascontiguousarray(np.asarray(Wk, np.float32)[:, sl]),
                "wv": np.ascontiguousarray(np.asarray(Wv, np.float32)[:, sl]),
                "wproj": np.ascontiguousarray(np.asarray(Wproj, np.float32)[sl, :]),
                "bpre": np.asarray(bpre, np.float32),
                "pi": np.asarray(pi, np.float32).reshape(1, 1),
            }
        )
    from concourse.bass_utils import run_bass_kernel_spmd

    res = run_bass_kernel_spmd(nc, in_maps, list(range(NCORES)))
    y = np.empty((B, N, C), np.float32)
    for b in range(B):
        y[b] = (
            res.results[2 * b]["y"]
            + res.results[2 * b + 1]["y"]
            + x[b]
            + np.asarray(bproj, np.float32)[None, :]
        )
    return y
